# revision 1
# baseline (speedup 1.0000x reference)
"""Depthwise-masked 3x3 conv (eye-masked dense conv) on 8 TRN2 NeuronCores.

Problem: x (2,16,256,64,64) fp32, W (256,256,3,3) fp32; the reference masks W
with eye(C) so only W[c,c,:,:] survives -> depthwise 3x3 "same" conv.

Strategy (per core; data-parallel over the 32 (s,b) samples -> 4 samples/core):
  - channels on partitions: work tile = (sample, channel-block of 128) ->
    bf16 x tile [128, 64, 64] via SWDGE cast-DMA; 8 work tiles per core.
  - PE path: taps as diagonal-stationary bf16 matmuls accumulating in fp32
    PSUM per 512-element bank chunk; boundaries via clipped access patterns
    (bf16 ifmaps allow the odd-width views fp32r rejects).
  - DVE path: per tap, tensor_scalar multiply (4x perf mode, dense from
    offset 0) into a tmp, then tensor_tensor add (2x) with the dh/dw shift
    absorbed into the add's read view; X1 (W-shifted padded copy, built by
    ACT) keeps the column-shifted reads 4B-aligned.
  - hybrid tiles split taps: PE takes the 6 column-shifted taps, DVE takes
    the 3 dw=0 taps and adds its partial onto the evicted PSUM result.
  - rel tolerance is 2e-2; bf16 everywhere lands ~3.5e-3.
"""

import os
from contextlib import ExitStack

import numpy as np
import ml_dtypes

import concourse.bass as bass
import concourse.tile as tile
from concourse import bacc, mybir
from concourse.bass_utils import run_bass_kernel_spmd

S, B, C, H, W_SP = 2, 16, 256, 64, 64
N_CORES = 8
N_SAMPLES = S * B                      # 32
SPC = N_SAMPLES // N_CORES             # 4 samples per core
NBLK = C // 128                        # 2 channel blocks
N_TILES = SPC * NBLK                   # 8 work tiles per core
WPAD = W_SP + 2                        # 66: zero col, 64 data cols, zero col
ROWS_PER_CHUNK = 8                     # 512 fp32 = one PSUM bank
HALF_CHUNKS = 4                        # chunks per half tile (4 banks)
HALF_ROWS = HALF_CHUNKS * ROWS_PER_CHUNK  # 32
HSPLIT = HALF_ROWS + 2                 # x0 half-DMA split row

# center tap first: the start=True matmul covers the full bank
TAPS = [(0, 0), (-1, -1), (-1, 0), (-1, 1), (0, -1), (0, 1), (1, -1), (1, 0), (1, 1)]
DW0_TAPS = [(0, 0), (-1, 0), (1, 0)]                       # DVE side of hybrids
DWX_TAPS = [(-1, -1), (-1, 1), (0, -1), (0, 1), (1, -1), (1, 1)]  # PE side

_DVE_TILES_DEFAULT = "1,5"
_HYB_TILES_DEFAULT = "2,6"
DVE_TILES = frozenset(
    int(v) for v in os.environ.get("KERNEL_DVE_TILES", _DVE_TILES_DEFAULT).split(",")
    if v != ""
)
HYB_TILES = frozenset(
    int(v) for v in os.environ.get("KERNEL_HYB_TILES", _HYB_TILES_DEFAULT).split(",")
    if v != ""
)

F32 = mybir.dt.float32
BF16 = mybir.dt.bfloat16


def _tap_slot(g, t):
    return (g % NBLK) * 9 + t


def _emit_pe_taps(nc, psum, g, half, x0, wd_sb, taps):
    """Diag-matmul the given taps for rows [32*half, ...) into psum.

    Clipped access patterns handle all boundaries: start=True clears the
    whole bank's has_written bits, each element's first writer overwrites,
    later ones accumulate — order independent, so partial-width taps are
    fine as long as every element is covered by some tap.
    """
    for i, (dh, dw) in enumerate(taps):
        t = TAPS.index((dh, dw))
        lhsT = wd_sb[:, _tap_slot(g, t) * 128:(_tap_slot(g, t) + 1) * 128]
        co0 = max(0, -dw)              # first valid output col
        co1 = W_SP - max(0, dw)        # one past last valid output col
        for q in range(HALF_CHUNKS):
            h0 = half * HALF_ROWS + q * ROWS_PER_CHUNK
            a = max(h0, -dh)                      # first valid output row
            b = min(h0 + ROWS_PER_CHUNK, H - dh)  # one past last valid row
            rhs = x0[:, a + dh:b + dh, co0 + dw:co1 + dw]
            out_ap = psum[:, q, a - h0:b - h0, co0:co1]
            nc.tensor.matmul(out_ap, lhsT, rhs,
                             start=(i == 0), stop=(i == len(taps) - 1))


def _load_x0(nc, tc, x0_pool, g, x_d):
    x0 = x0_pool.tile([128, H, W_SP], BF16, tag="x0")
    # stagger issue so early tiles (and output stores) get full DMA bandwidth
    with tc.tile_wait_until(g * 0.007):
        nc.gpsimd.dma_start(x0[:, 0:HSPLIT, :],
                            x_d[g * 128:(g + 1) * 128, 0:HSPLIT, :])  # ->bf16
        nc.gpsimd.dma_start(x0[:, HSPLIT:H, :],
                            x_d[g * 128:(g + 1) * 128, HSPLIT:H, :])
    return x0


def _emit_pe_tile(nc, tc, pools, g, x_d, out_d, wd_sb, x0_pool):
    psum_pool, osb_pool, osbf_pool = pools
    x0 = _load_x0(nc, tc, x0_pool, g, x_d)
    for half in range(2):
        psum = psum_pool.tile([128, HALF_CHUNKS, ROWS_PER_CHUNK, W_SP], F32,
                              tag="psum")
        _emit_pe_taps(nc, psum, g, half, x0, wd_sb, TAPS)
        # f32 eviction + HWDGE store (keeps Sync busy, GpSimd free)
        osb = osbf_pool.tile([128, HALF_CHUNKS * 512], F32, tag="osbf")
        nc.scalar.copy(osb[:], psum[:, :, :, :])
        nc.sync.dma_start(
            out_d[g * 128:(g + 1) * 128,
                  half * HALF_ROWS:(half + 1) * HALF_ROWS, :],
            osb[:],
        )


def _dve_dw0_partial(nc, tmp_pool, part_pool, g, x0, wv_sb):
    """part = sum of the three dw=0 taps (bf16, dense 4x/2x ops only)."""
    cb9 = (g % NBLK) * 9
    part = part_pool.tile([128, H, W_SP], BF16, tag="part")
    # the multiplies run on ACT (activation copy with per-partition scale),
    # freeing DVE for the adds
    nc.scalar.mul(part[:], x0[:], wv_sb[:, cb9:cb9 + 1])
    for dh in (-1, 1):
        t = TAPS.index((dh, 0))
        wv = wv_sb[:, cb9 + t:cb9 + t + 1]
        tmp = tmp_pool.tile([128, H, W_SP], BF16, tag="tmp")
        nc.scalar.mul(tmp[:], x0[:], wv)
        oa = max(0, -dh)
        ob = H - max(0, dh)
        nc.vector.tensor_tensor(part[:, oa:ob, :], part[:, oa:ob, :],
                                tmp[:, oa + dh:ob + dh, :],
                                op=mybir.AluOpType.add)
    return part


def _emit_hyb_tile(nc, tc, pools, g, x_d, out_d, wd_sb, wv_sb, x0_pool,
                   tmp_pool, part_pool):
    psum_pool, osb_pool, osbf_pool = pools
    x0 = _load_x0(nc, tc, x0_pool, g, x_d)
    part = _dve_dw0_partial(nc, tmp_pool, part_pool, g, x0, wv_sb)
    for half in range(2):
        psum = psum_pool.tile([128, HALF_CHUNKS, ROWS_PER_CHUNK, W_SP], F32,
                              tag="psum")
        _emit_pe_taps(nc, psum, g, half, x0, wd_sb, DWX_TAPS)
        osb = osb_pool.tile([128, HALF_ROWS, W_SP], BF16, tag="osb")
        nc.scalar.copy(osb[:, :, :], psum[:, :, :, :])
        pv = part[:, half * HALF_ROWS:(half + 1) * HALF_ROWS, :]
        nc.vector.tensor_tensor(osb[:, :, :], osb[:, :, :], pv,
                                op=mybir.AluOpType.add)
        nc.gpsimd.dma_start(
            out_d[g * 128:(g + 1) * 128,
                  half * HALF_ROWS:(half + 1) * HALF_ROWS, :],
            osb[:, :, :],  # bf16 -> f32 cast store
        )


def _emit_dve_tile(nc, tc, pools, g, x_d, out_d, wv_sb, zb_sb, x0_pool):
    """bf16 DVE path for the whole work tile g.

    scalar_tensor_tensor has no fast uop (always 1x), so each tap is
    tensor_scalar (4x mode, dense from offset 0) into a tmp followed by a
    tensor_tensor add (2x) whose read view carries the dh/dw shift (4B
    alignment is enough for 2x).
    """
    x1_pool, tmp_pool, tmp6_pool, odve_pool = pools
    cb = g % NBLK

    x0 = _load_x0(nc, tc, x0_pool, g, x_d)

    # X1[h, 1+w] = x[h, w]; zero cols 0 and 65.  dw=-1 reads cols 0:64
    # (byte offset 0), dw=+1 reads cols 2:66 (byte offset 4).
    x1 = x1_pool.tile([128, H, WPAD], BF16, tag="x1")
    nc.scalar.copy(x1[:, :, 0:1], zb_sb[:])
    nc.scalar.copy(x1[:, :, WPAD - 1:WPAD], zb_sb[:])
    nc.scalar.copy(x1[:, 0:HSPLIT, 1:1 + W_SP], x0[:, 0:HSPLIT, :])
    nc.scalar.copy(x1[:, HSPLIT:H, 1:1 + W_SP], x0[:, HSPLIT:H, :])

    odve = odve_pool.tile([128, H, W_SP], BF16, tag="odve")
    nc.vector.tensor_scalar(odve[:], x0[:], wv_sb[:, cb * 9:cb * 9 + 1], None,
                            mybir.AluOpType.mult)

    for t, (dh, dw) in enumerate(TAPS[1:], start=1):
        wv = wv_sb[:, cb * 9 + t:cb * 9 + t + 1]
        oa = max(0, -dh)          # first valid output row
        ob = H - max(0, dh)       # one past last valid output row
        if dw == 0:
            tmp = tmp_pool.tile([128, H, W_SP], BF16, tag="tmp")
            nc.scalar.mul(tmp[:], x0[:], wv)  # on ACT: DVE keeps only the add
            in_v = tmp[:, oa + dh:ob + dh, :]
        else:
            tmp6 = tmp6_pool.tile([128, H, WPAD], BF16, tag="tmp6")
            nc.vector.tensor_scalar(tmp6[:], x1[:], wv, None,
                                    mybir.AluOpType.mult)
            col0 = 0 if dw == -1 else 2
            in_v = tmp6[:, oa + dh:ob + dh, col0:col0 + W_SP]
        out_v = odve[:, oa:ob, :]
        nc.vector.tensor_tensor(out_v, out_v, in_v, op=mybir.AluOpType.add)
    nc.gpsimd.dma_start(out_d[g * 128:(g + 1) * 128, :, :], odve[:])  # ->f32


def _build_program(dve_tiles, hyb_tiles):
    nc = bacc.Bacc("TRN2", target_bir_lowering=False, debug=False)
    x_d = nc.dram_tensor("x", [SPC * C, H, W_SP], F32, kind="ExternalInput").ap()
    wd_d = nc.dram_tensor("wd", [128, NBLK * 9 * 128], BF16, kind="ExternalInput").ap()
    wv_d = nc.dram_tensor("wv", [128, NBLK * 9], F32, kind="ExternalInput").ap()
    out_d = nc.dram_tensor("out", [SPC * C, H, W_SP], F32, kind="ExternalOutput").ap()

    with tile.TileContext(nc) as tc:
        with ExitStack() as ctx:
            const_pool = ctx.enter_context(tc.tile_pool(name="const", bufs=1))
            wd_sb = const_pool.tile([128, NBLK * 9 * 128], BF16)
            nc.sync.dma_start(wd_sb[:], wd_d[:])
            wv_sb = const_pool.tile([128, NBLK * 9], F32)
            nc.sync.dma_start(wv_sb[:], wv_d[:])
            zb_sb = const_pool.tile([128, H, 1], BF16)
            nc.vector.memset(zb_sb[:], 0.0)

            psum_pool = ctx.enter_context(tc.tile_pool(name="psum", bufs=2, space="PSUM"))
            osb_pool = ctx.enter_context(tc.tile_pool(name="osb", bufs=3))
            osbf_pool = ctx.enter_context(tc.tile_pool(name="osbf", bufs=4))
            x0_pool = ctx.enter_context(tc.tile_pool(name="x0", bufs=5))
            x1_pool = ctx.enter_context(tc.tile_pool(name="x1", bufs=2))
            tmp_pool = ctx.enter_context(tc.tile_pool(name="tmp", bufs=2))
            tmp6_pool = ctx.enter_context(tc.tile_pool(name="tmp6", bufs=2))
            part_pool = ctx.enter_context(tc.tile_pool(name="part", bufs=2))
            odve_pool = ctx.enter_context(tc.tile_pool(name="odve", bufs=2))
            pe_pools = (psum_pool, osb_pool, osbf_pool)
            dve_pools = (x1_pool, tmp_pool, tmp6_pool, odve_pool)

            for g in range(N_TILES):
                if g in dve_tiles:
                    _emit_dve_tile(nc, tc, dve_pools, g, x_d, out_d, wv_sb,
                                   zb_sb, x0_pool)
                elif g in hyb_tiles:
                    _emit_hyb_tile(nc, tc, pe_pools, g, x_d, out_d, wd_sb,
                                   wv_sb, x0_pool, tmp_pool, part_pool)
                else:
                    _emit_pe_tile(nc, tc, pe_pools, g, x_d, out_d, wd_sb, x0_pool)
    nc.compile()
    return nc


_prog_cache = {}


def _get_program():
    key = (DVE_TILES, HYB_TILES)
    if key not in _prog_cache:
        _prog_cache[key] = _build_program(DVE_TILES, HYB_TILES)
    return _prog_cache[key]


def _host_weights(W):
    wdiag = W[np.arange(C), np.arange(C)]          # [256, 3, 3]
    wd_host = np.zeros((128, NBLK * 9, 128), dtype=np.float32)
    wv_host = np.zeros((128, NBLK * 9), dtype=np.float32)
    r = np.arange(128)
    for cb in range(NBLK):
        for t, (dh, dw) in enumerate(TAPS):
            wd_host[r, cb * 9 + t, r] = wdiag[cb * 128 + r, dh + 1, dw + 1]
            wv_host[r, cb * 9 + t] = wdiag[cb * 128 + r, dh + 1, dw + 1]
    return wd_host.reshape(128, NBLK * 9 * 128).astype(ml_dtypes.bfloat16), wv_host


def _in_maps(x, W):
    wd_host, wv_host = _host_weights(W)
    xs = x.reshape(N_SAMPLES, C, H, W_SP)
    return [
        {
            "x": np.ascontiguousarray(xs[i * SPC:(i + 1) * SPC]).reshape(SPC * C, H, W_SP),
            "wd": wd_host,
            "wv": wv_host,
        }
        for i in range(N_CORES)
    ]


def kernel(x: np.ndarray, W: np.ndarray) -> np.ndarray:
    x = np.ascontiguousarray(x, dtype=np.float32)
    W = np.ascontiguousarray(W, dtype=np.float32)
    assert x.shape == (S, B, C, H, W_SP)
    assert W.shape == (C, C, 3, 3)

    nc = _get_program()
    res = run_bass_kernel_spmd(nc, _in_maps(x, W), core_ids=list(range(N_CORES)))
    out = np.concatenate(
        [res.results[i]["out"].reshape(SPC, C, H, W_SP) for i in range(N_CORES)], axis=0
    )
    return out.reshape(S, B, C, H, W_SP)



# revision 2
# speedup vs baseline: 1.0776x; 1.0776x over previous
"""Depthwise-masked 3x3 conv (eye-masked dense conv) on 8 TRN2 NeuronCores.

Problem: x (2,16,256,64,64) fp32, W (256,256,3,3) fp32; the reference masks W
with eye(C) so only W[c,c,:,:] survives -> depthwise 3x3 "same" conv.

v2 strategy (per core; data-parallel over the 32 (s,b) samples -> 4/core):
  - bf16 in HBM both directions (host casts x -> bf16, upcasts out -> f32):
    halves HBM traffic vs fp32, DMA floor ~47us/core at ~358GB/s.
  - work tile = (sample, 128-channel block): x0p [128, 66, 64] bf16 with
    zero pad rows 0/65 (kills row-clipping everywhere).
  - PE: the 6 column-shifted taps as diagonal-stationary bf16 matmuls into
    fp32 PSUM (4-bank halves, 512-elem bank chunks, clipped col views).
  - ACT: PSUM -> bf16 SBUF eviction (1x, (172+FD)/1.2) + optionally some of
    the dw=0 tap multiplies.
  - DVE: dw=0 tap products via tensor_scalar (4x on dense bf16) and the
    3 accumulate adds per half via tensor_tensor (2x) straight onto the
    evicted bf16 result (no separate accumulator, no 1x psum merge).
  - rel tolerance is 2e-2; this precision structure lands ~3e-3.
"""

import os
from contextlib import ExitStack

import numpy as np
import ml_dtypes

import concourse.bass as bass
import concourse.tile as tile
from concourse import bacc, mybir
from concourse.bass_utils import run_bass_kernel_spmd

S, B, C, H, W_SP = 2, 16, 256, 64, 64
N_CORES = 8
N_SAMPLES = S * B                      # 32
SPC = N_SAMPLES // N_CORES             # 4 samples per core
NBLK = C // 128                        # 2 channel blocks
N_TILES = SPC * NBLK                   # 8 work tiles per core
HP = H + 2                             # 66: zero row, 64 data rows, zero row
ROWS_PER_CHUNK = 8                     # 512 fp32 = one PSUM bank
HALF_CHUNKS = 4                        # chunks per half tile (4 banks)
HALF_ROWS = HALF_CHUNKS * ROWS_PER_CHUNK  # 32
HSPLIT = HALF_ROWS + 2                 # x data rows in first in-DMA half

TAPS = [(0, 0), (-1, -1), (-1, 0), (-1, 1), (0, -1), (0, 1), (1, -1), (1, 0), (1, 1)]
DVE_TAPS = [(0, 0), (-1, 0), (1, 0)]                       # dw=0: DVE/ACT side
PE_TAPS = [(-1, -1), (-1, 1), (0, -1), (0, 1), (1, -1), (1, 1)]  # PE side

# tunables
ACT_MULS = int(os.environ.get("KERNEL_ACT_MULS", "1"))   # dw0 muls on ACT per tile (0-3)
STAGGER = float(os.environ.get("KERNEL_STAGGER", "0.007"))

F32 = mybir.dt.float32
BF16 = mybir.dt.bfloat16


def _slot(cb, tap):
    return cb * 9 + TAPS.index(tap)


def _emit_tile(nc, tc, g, x_d, out_d, wd_sb, wv_sb, pools):
    x0_pool, tmp_pool, osb_pool, psum_pool = pools
    cb = g % NBLK

    x0p = x0_pool.tile([128, HP, W_SP], BF16, tag="x0p")
    # stagger issue so early tiles (and output stores) get full DMA bandwidth
    with tc.tile_wait_until(g * STAGGER):
        nc.vector.memset(x0p[:, 0:1, :], 0.0)
        nc.vector.memset(x0p[:, HP - 1:HP, :], 0.0)
        nc.sync.dma_start(x0p[:, 1:1 + HSPLIT, :],
                          x_d[g * 128:(g + 1) * 128, 0:HSPLIT, :])
        nc.sync.dma_start(x0p[:, 1 + HSPLIT:1 + H, :],
                          x_d[g * 128:(g + 1) * 128, HSPLIT:H, :])

    # dw=0 tap products over the full padded tile (dense bf16 -> DVE 4x)
    tmps = []
    for j, (dh, _) in enumerate(DVE_TAPS):
        wv = wv_sb[:, _slot(cb, (dh, 0)):_slot(cb, (dh, 0)) + 1]
        tmp = tmp_pool.tile([128, HP, W_SP], BF16, tag="tmp")
        if j < ACT_MULS:
            nc.scalar.mul(tmp[:], x0p[:], wv)
        else:
            nc.vector.tensor_scalar(tmp[:], x0p[:], wv, None,
                                    mybir.AluOpType.mult)
        tmps.append((dh, tmp))

    osb = osb_pool.tile([128, H, W_SP], BF16, tag="osb")
    for half in range(2):
        psum = psum_pool.tile([128, HALF_CHUNKS, ROWS_PER_CHUNK, W_SP], F32,
                              tag="psum")
        # 6 column-shifted taps; padded rows -> no row clipping. Clipped col
        # views: start=True on tap 0 clears the bank; each element's first
        # writer overwrites, later ones accumulate (order independent).
        for i, (dh, dw) in enumerate(PE_TAPS):
            s = _slot(cb, (dh, dw))
            lhsT = wd_sb[:, s * 128:(s + 1) * 128]
            co0 = max(0, -dw)
            co1 = W_SP - max(0, dw)
            for q in range(HALF_CHUNKS):
                r = 1 + dh + half * HALF_ROWS + q * ROWS_PER_CHUNK
                rhs = x0p[:, r:r + ROWS_PER_CHUNK, co0 + dw:co1 + dw]
                nc.tensor.matmul(psum[:, q, :, co0:co1], lhsT, rhs,
                                 start=(i == 0), stop=(i == len(PE_TAPS) - 1))
        ov = osb[:, half * HALF_ROWS:(half + 1) * HALF_ROWS, :]
        nc.scalar.copy(ov, psum[:, :, :, :])
        for dh, tmp in tmps:
            tv = tmp[:, 1 + dh + half * HALF_ROWS:
                     1 + dh + (half + 1) * HALF_ROWS, :]
            nc.vector.tensor_tensor(ov, ov, tv, op=mybir.AluOpType.add)
        nc.gpsimd.dma_start(
            out_d[g * 128:(g + 1) * 128,
                  half * HALF_ROWS:(half + 1) * HALF_ROWS, :],
            ov,
        )


def _build_program(act_muls, stagger):
    nc = bacc.Bacc("TRN2", target_bir_lowering=False, debug=False)
    x_d = nc.dram_tensor("x", [SPC * C, H, W_SP], BF16, kind="ExternalInput").ap()
    wd_d = nc.dram_tensor("wd", [128, NBLK * 9 * 128], BF16, kind="ExternalInput").ap()
    wv_d = nc.dram_tensor("wv", [128, NBLK * 9], F32, kind="ExternalInput").ap()
    out_d = nc.dram_tensor("out", [SPC * C, H, W_SP], BF16, kind="ExternalOutput").ap()

    with tile.TileContext(nc) as tc:
        with ExitStack() as ctx:
            const_pool = ctx.enter_context(tc.tile_pool(name="const", bufs=1))
            wd_sb = const_pool.tile([128, NBLK * 9 * 128], BF16)
            nc.sync.dma_start(wd_sb[:], wd_d[:])
            wv_sb = const_pool.tile([128, NBLK * 9], F32)
            nc.sync.dma_start(wv_sb[:], wv_d[:])

            psum_pool = ctx.enter_context(tc.tile_pool(name="psum", bufs=2, space="PSUM"))
            x0_pool = ctx.enter_context(tc.tile_pool(name="x0", bufs=4))
            tmp_pool = ctx.enter_context(tc.tile_pool(name="tmp", bufs=6))
            osb_pool = ctx.enter_context(tc.tile_pool(name="osb", bufs=3))
            pools = (x0_pool, tmp_pool, osb_pool, psum_pool)

            for g in range(N_TILES):
                _emit_tile(nc, tc, g, x_d, out_d, wd_sb, wv_sb, pools)
    nc.compile()
    return nc


_prog_cache = {}


def _get_program():
    key = (ACT_MULS, STAGGER)
    if key not in _prog_cache:
        _prog_cache[key] = _build_program(ACT_MULS, STAGGER)
    return _prog_cache[key]


def _host_weights(W):
    wdiag = W[np.arange(C), np.arange(C)]          # [256, 3, 3]
    wd_host = np.zeros((128, NBLK * 9, 128), dtype=np.float32)
    wv_host = np.zeros((128, NBLK * 9), dtype=np.float32)
    r = np.arange(128)
    for cb in range(NBLK):
        for t, (dh, dw) in enumerate(TAPS):
            wd_host[r, cb * 9 + t, r] = wdiag[cb * 128 + r, dh + 1, dw + 1]
            wv_host[r, cb * 9 + t] = wdiag[cb * 128 + r, dh + 1, dw + 1]
    return wd_host.reshape(128, NBLK * 9 * 128).astype(ml_dtypes.bfloat16), wv_host


def _in_maps(x, W):
    wd_host, wv_host = _host_weights(np.asarray(W, dtype=np.float32))
    xb = np.asarray(x, dtype=np.float32).astype(ml_dtypes.bfloat16)
    xs = xb.reshape(N_SAMPLES, C, H, W_SP)
    return [
        {
            "x": np.ascontiguousarray(xs[i * SPC:(i + 1) * SPC]).reshape(SPC * C, H, W_SP),
            "wd": wd_host,
            "wv": wv_host,
        }
        for i in range(N_CORES)
    ]


def kernel(x: np.ndarray, W: np.ndarray) -> np.ndarray:
    x = np.ascontiguousarray(x, dtype=np.float32)
    W = np.ascontiguousarray(W, dtype=np.float32)
    assert x.shape == (S, B, C, H, W_SP)
    assert W.shape == (C, C, 3, 3)

    nc = _get_program()
    res = run_bass_kernel_spmd(nc, _in_maps(x, W), core_ids=list(range(N_CORES)))
    out = np.concatenate(
        [res.results[i]["out"].reshape(SPC, C, H, W_SP).astype(np.float32)
         for i in range(N_CORES)],
        axis=0,
    )
    return out.reshape(S, B, C, H, W_SP)


# revision 3
# speedup vs baseline: 1.1009x; 1.0216x over previous
"""Depthwise-masked 3x3 conv (eye-masked dense conv) on 8 TRN2 NeuronCores.

Problem: x (2,16,256,64,64) fp32, W (256,256,3,3) fp32; the reference masks W
with eye(C) so only W[c,c,:,:] survives -> depthwise 3x3 "same" conv.

v3 strategy (per core; data-parallel over the 32 (s,b) samples -> 4/core):
  - bf16 in HBM both directions (host casts x -> bf16, upcasts out -> f32):
    halves HBM traffic vs fp32, DMA floor ~47us/core at ~358GB/s.
  - work tile = (sample, 128-channel block): x0p [128, 66, 64] bf16 with
    zero pad rows 0/65 (kills row-clipping everywhere).
  - PE: the 6 column-shifted taps as diagonal-stationary bf16 matmuls into
    fp32 PSUM (4-bank halves, 512-elem bank chunks, clipped col views).
  - DVE/ACT: the 3 dw=0 taps: per-tap products (tensor_scalar 4x / ACT mul),
    pre-summed into one S2 tile with full-tile 2x adds OFF the critical
    path; after the ACT psum->bf16 evict only ONE 2x add per half remains
    before the store.
  - head: tile-0 x loaded in row-chunks before the (split) weight loads so
    the first matmul issues ~3us in; tail: last tile evict/add/store at
    2-chunk granularity.
"""

import os
from contextlib import ExitStack

import numpy as np
import ml_dtypes

import concourse.bass as bass
import concourse.tile as tile
from concourse import bacc, mybir
from concourse.bass_utils import run_bass_kernel_spmd

S, B, C, H, W_SP = 2, 16, 256, 64, 64
N_CORES = 8
N_SAMPLES = S * B                      # 32
SPC = N_SAMPLES // N_CORES             # 4 samples per core
NBLK = C // 128                        # 2 channel blocks
N_TILES = SPC * NBLK                   # 8 work tiles per core
HP = H + 2                             # 66: zero row, 64 data rows, zero row
ROWS_PER_CHUNK = 8                     # 512 fp32 = one PSUM bank
HALF_CHUNKS = 4                        # chunks per half tile (4 banks)
HALF_ROWS = HALF_CHUNKS * ROWS_PER_CHUNK  # 32
HSPLIT = HALF_ROWS + 2                 # x data rows in first in-DMA half

TAPS = [(0, 0), (-1, -1), (-1, 0), (-1, 1), (0, -1), (0, 1), (1, -1), (1, 0), (1, 1)]
DVE_TAPS = [(0, 0), (-1, 0), (1, 0)]                       # dw=0: DVE/ACT side
PE_TAPS = [(-1, -1), (-1, 1), (0, -1), (0, 1), (1, -1), (1, 1)]  # PE side

ACT_MULS = int(os.environ.get("KERNEL_ACT_MULS", "1"))   # dw0 muls on ACT per tile (0-3)
STAGGER = float(os.environ.get("KERNEL_STAGGER", "0.007"))

F32 = mybir.dt.float32
BF16 = mybir.dt.bfloat16


def _slot(cb, tap):
    return cb * 9 + TAPS.index(tap)


def _load_x(nc, tc, x0_pool, g, x_d):
    """Padded x tile; tile 0 loads in finer chunks to cut the pipeline head."""
    x0p = x0_pool.tile([128, HP, W_SP], BF16, tag="x0p")
    splits = (10, 18, 26, HSPLIT, H) if g == 0 else (HSPLIT, H)
    with tc.tile_wait_until(g * STAGGER):
        nc.vector.memset(x0p[:, 0:1, :], 0.0)
        nc.vector.memset(x0p[:, HP - 1:HP, :], 0.0)
        r0 = 0
        for r1 in splits:
            nc.sync.dma_start(x0p[:, 1 + r0:1 + r1, :],
                              x_d[g * 128:(g + 1) * 128, r0:r1, :])
            r0 = r1
    return x0p


def _emit_dw0_sum(nc, g, x0p, wv_sb, tmp_pool, s2_pool):
    """S2 = sum of the 3 dw=0 tap products, aligned to output rows 0..63.

    All full-tile ops off the evict->store critical path; the adds' shifted
    read views keep even element offsets (4B-aligned) so tensor_tensor
    stays in 2x mode.
    """
    cb = g % NBLK
    tmps = []
    for j, (dh, _) in enumerate(DVE_TAPS):
        s = _slot(cb, (dh, 0))
        wv = wv_sb[:, s:s + 1]
        tmp = tmp_pool.tile([128, HP, W_SP], BF16, tag="tmp")
        if j < ACT_MULS:
            nc.scalar.mul(tmp[:], x0p[:], wv)
        else:
            nc.vector.tensor_scalar(tmp[:], x0p[:], wv, None,
                                    mybir.AluOpType.mult)
        tmps.append((dh, tmp))
    s2 = s2_pool.tile([128, H, W_SP], BF16, tag="s2")
    (dh0, t0), (dh1, t1), (dh2, t2) = tmps
    nc.vector.tensor_tensor(s2[:], t0[:, 1 + dh0:1 + dh0 + H, :],
                            t1[:, 1 + dh1:1 + dh1 + H, :],
                            op=mybir.AluOpType.add)
    nc.vector.tensor_tensor(s2[:], s2[:], t2[:, 1 + dh2:1 + dh2 + H, :],
                            op=mybir.AluOpType.add)
    return s2


def _emit_pe_half(nc, g, half, x0p, wd_sb, psum):
    """6 column-shifted taps into psum for rows [32*half, 32*half+32).

    Padded rows -> no row clipping. Clipped col views: start=True on tap 0
    clears the bank; each element's first writer overwrites, later ones
    accumulate (order independent).
    """
    cb = g % NBLK
    for i, (dh, dw) in enumerate(PE_TAPS):
        s = _slot(cb, (dh, dw))
        lhsT = wd_sb[:, s * 128:(s + 1) * 128]
        co0 = max(0, -dw)
        co1 = W_SP - max(0, dw)
        for q in range(HALF_CHUNKS):
            r = 1 + dh + half * HALF_ROWS + q * ROWS_PER_CHUNK
            rhs = x0p[:, r:r + ROWS_PER_CHUNK, co0 + dw:co1 + dw]
            nc.tensor.matmul(psum[:, q, :, co0:co1], lhsT, rhs,
                             start=(i == 0), stop=(i == len(PE_TAPS) - 1))


def _emit_tile(nc, tc, g, x_d, out_d, wd_sb, wv_sb, pools):
    x0_pool, tmp_pool, s2_pool, osb_pool, psum_pool = pools
    x0p = _load_x(nc, tc, x0_pool, g, x_d)
    s2 = _emit_dw0_sum(nc, g, x0p, wv_sb, tmp_pool, s2_pool)
    osb = osb_pool.tile([128, H, W_SP], BF16, tag="osb")
    last = g == N_TILES - 1
    for half in range(2):
        psum = psum_pool.tile([128, HALF_CHUNKS, ROWS_PER_CHUNK, W_SP], F32,
                              tag="psum")
        _emit_pe_half(nc, g, half, x0p, wd_sb, psum)
        # evict + single add + store; finer grain on the last half to cut
        # the pipeline tail
        pieces = 2 if (last and half == 1) else 1
        rows_pp = HALF_ROWS // pieces
        for p in range(pieces):
            r0 = half * HALF_ROWS + p * rows_pp
            ov = osb[:, r0:r0 + rows_pp, :]
            nc.scalar.copy(ov, psum[:, p * (HALF_CHUNKS // pieces):
                                    (p + 1) * (HALF_CHUNKS // pieces), :, :])
            nc.vector.tensor_tensor(ov, ov, s2[:, r0:r0 + rows_pp, :],
                                    op=mybir.AluOpType.add)
            nc.gpsimd.dma_start(
                out_d[g * 128:(g + 1) * 128, r0:r0 + rows_pp, :], ov)


def _build_program(act_muls, stagger):
    nc = bacc.Bacc("TRN2", target_bir_lowering=False, debug=False)
    x_d = nc.dram_tensor("x", [SPC * C, H, W_SP], BF16, kind="ExternalInput").ap()
    wd_d = nc.dram_tensor("wd", [128, NBLK * 9 * 128], BF16, kind="ExternalInput").ap()
    wv_d = nc.dram_tensor("wv", [128, NBLK * 9], F32, kind="ExternalInput").ap()
    out_d = nc.dram_tensor("out", [SPC * C, H, W_SP], BF16, kind="ExternalOutput").ap()

    # cb=0 PE-tap weight slots first so the first matmuls unblock early
    wd_head = sorted(_slot(0, t) for t in PE_TAPS)

    with tile.TileContext(nc) as tc:
        with ExitStack() as ctx:
            const_pool = ctx.enter_context(tc.tile_pool(name="const", bufs=1))
            wd_sb = const_pool.tile([128, NBLK * 9 * 128], BF16)
            wv_sb = const_pool.tile([128, NBLK * 9], F32)
            nc.sync.dma_start(wv_sb[:], wv_d[:])
            done = set()
            for s in wd_head:
                nc.gpsimd.dma_start(wd_sb[:, s * 128:(s + 1) * 128],
                                    wd_d[:, s * 128:(s + 1) * 128])
                done.add(s)
            rest = [s for s in range(NBLK * 9) if s not in done]
            # remaining slots are contiguous runs; batch them
            r0 = 0
            while r0 < len(rest):
                r1 = r0
                while r1 + 1 < len(rest) and rest[r1 + 1] == rest[r1] + 1:
                    r1 += 1
                a, b = rest[r0], rest[r1] + 1
                nc.gpsimd.dma_start(wd_sb[:, a * 128:b * 128],
                                    wd_d[:, a * 128:b * 128])
                r0 = r1 + 1

            psum_pool = ctx.enter_context(tc.tile_pool(name="psum", bufs=2, space="PSUM"))
            x0_pool = ctx.enter_context(tc.tile_pool(name="x0", bufs=4))
            tmp_pool = ctx.enter_context(tc.tile_pool(name="tmp", bufs=4))
            s2_pool = ctx.enter_context(tc.tile_pool(name="s2", bufs=3))
            osb_pool = ctx.enter_context(tc.tile_pool(name="osb", bufs=3))
            pools = (x0_pool, tmp_pool, s2_pool, osb_pool, psum_pool)

            for g in range(N_TILES):
                _emit_tile(nc, tc, g, x_d, out_d, wd_sb, wv_sb, pools)
    nc.compile()
    return nc


_prog_cache = {}


def _get_program():
    key = (ACT_MULS, STAGGER)
    if key not in _prog_cache:
        _prog_cache[key] = _build_program(ACT_MULS, STAGGER)
    return _prog_cache[key]


def _host_weights(W):
    wdiag = W[np.arange(C), np.arange(C)]          # [256, 3, 3]
    wd_host = np.zeros((128, NBLK * 9, 128), dtype=np.float32)
    wv_host = np.zeros((128, NBLK * 9), dtype=np.float32)
    r = np.arange(128)
    for cb in range(NBLK):
        for t, (dh, dw) in enumerate(TAPS):
            wd_host[r, cb * 9 + t, r] = wdiag[cb * 128 + r, dh + 1, dw + 1]
            wv_host[r, cb * 9 + t] = wdiag[cb * 128 + r, dh + 1, dw + 1]
    return wd_host.reshape(128, NBLK * 9 * 128).astype(ml_dtypes.bfloat16), wv_host


def _in_maps(x, W):
    wd_host, wv_host = _host_weights(np.asarray(W, dtype=np.float32))
    xb = np.asarray(x, dtype=np.float32).astype(ml_dtypes.bfloat16)
    xs = xb.reshape(N_SAMPLES, C, H, W_SP)
    return [
        {
            "x": np.ascontiguousarray(xs[i * SPC:(i + 1) * SPC]).reshape(SPC * C, H, W_SP),
            "wd": wd_host,
            "wv": wv_host,
        }
        for i in range(N_CORES)
    ]


def kernel(x: np.ndarray, W: np.ndarray) -> np.ndarray:
    x = np.ascontiguousarray(x, dtype=np.float32)
    W = np.ascontiguousarray(W, dtype=np.float32)
    assert x.shape == (S, B, C, H, W_SP)
    assert W.shape == (C, C, 3, 3)

    nc = _get_program()
    res = run_bass_kernel_spmd(nc, _in_maps(x, W), core_ids=list(range(N_CORES)))
    out = np.concatenate(
        [res.results[i]["out"].reshape(SPC, C, H, W_SP).astype(np.float32)
         for i in range(N_CORES)],
        axis=0,
    )
    return out.reshape(S, B, C, H, W_SP)
